# revision 8
# baseline (speedup 1.0000x reference)
"""HOPE block kernel for 8 Trainium2 NeuronCores.

Sharding: 8 shards = (batch b in 0..3, sequence half in 0..1), 2048 tokens each.
Odd cores rebuild the mid-sequence linear-attention memory M from the first
half ("prefix"); even cores get a zero prefix (uniform SPMD program).

Precision plan (validated vs fp32 reference, rel err ~6e-4):
 - Attention path in fp16; accumulation and the M accumulator stay fp32.
 - CMS FFN in fp8(e4m3, clip 240) with DoubleRow matmuls; weights host
   prescaled by 64, descaled at PSUM eviction. ug lives entirely in SBUF.
 - Level-2 down-projection emits token-major output fused with the residual.

Pipeline plan:
 - LayerNorm gains are folded into the consuming weights host-side (exact);
   when the LN biases are nonzero a fallback un-folded build is used.
 - All own-token LN1+transposes run during phase A so phase B block
   boundaries never wait on the vector engine (keeps PE HAM-warm).
 - M updates batch 16 per-head deltas into one PSUM bank per chunk,
   committed in two half-copies so the next chunk's inter matmuls never
   stall on the update chain.
 - LN2 -> h2 transposes are software-pipelined by one block.
"""
import sys
if '/opt/trn_rl_repo' not in sys.path:
    sys.path.insert(0, '/opt/trn_rl_repo')

from contextlib import ExitStack
import numpy as np
import ml_dtypes

import concourse.bass as bass
import concourse.tile as tile
from concourse import mybir
from concourse.bass_utils import run_bass_kernel_spmd
from concourse.masks import make_identity

f32 = mybir.dt.float32
f16 = mybir.dt.float16
f8 = mybir.dt.float8e4
AF = mybir.ActivationFunctionType
ALU = mybir.AluOpType
DR = mybir.MatmulPerfMode.DoubleRow

DIM = 1024
HEADS = 16
HD = 64
B, S = 4, 4096
LEVELS = 3
HID = 4 * DIM
CHUNK = 128
EPS = 1e-5
P = 128

N_CORES = 8
T_OWN = S // 2
T_PRE = S // 2
BLK = 512
D_T = DIM // P      # 8 feature tiles
H_T = HID // P      # 32 hidden tiles
W_SCALE = 64.0      # fp8 weight prescale (host side)

MAX_WAITS = 1


def _split_multi_waits(nc, max_waits=MAX_WAITS):
    """Walrus encodes at most `max_waits` sem waits per instruction; split
    extras onto same-engine NOPs placed just before."""
    for f in nc.m.functions:
        for bb in f.blocks:
            insts = list(bb.instructions)
            if not any(
                i.sync_info and i.sync_info.on_wait and len(i.sync_info.on_wait) > max_waits
                for i in insts
            ):
                continue
            new = []
            for inst in insts:
                si = inst.sync_info
                waits = list(si.on_wait) if si and si.on_wait else []
                if len(waits) > max_waits:
                    head, rest = waits[:-max_waits], waits[-max_waits:]
                    while head:
                        chunk, head = head[:max_waits], head[max_waits:]
                        nop = mybir.InstNoOp(name=nc.get_next_instruction_name(), ins=[], outs=[])
                        nop.engine = inst.engine
                        nop.sync_info = mybir.SyncInfo(on_wait=chunk, on_update=[])
                        nc.register_instruction(nop, overwrite=True)
                        new.append(nop)
                    inst.sync_info = mybir.SyncInfo(
                        on_wait=rest, on_update=list(si.on_update) if si.on_update else [])
                new.append(inst)
            bb.instructions = new


def build_kernel(t_own=T_OWN, t_pre=T_PRE, fold_ln=True):
    nc = bass.Bass()

    x_own = nc.dram_tensor("x_own", [t_own, DIM], f32, kind="ExternalInput")
    x_pre = nc.dram_tensor("x_pre", [t_pre, DIM], f32, kind="ExternalInput")
    wq = nc.dram_tensor("wq", [DIM, DIM], f16, kind="ExternalInput")
    wk = nc.dram_tensor("wk", [DIM, DIM], f16, kind="ExternalInput")
    wv = nc.dram_tensor("wv", [DIM, DIM], f16, kind="ExternalInput")
    wo = nc.dram_tensor("wo", [DIM, DIM], f16, kind="ExternalInput")
    ln1_g = nc.dram_tensor("ln1_g", [DIM], f32, kind="ExternalInput")
    ln1_b = nc.dram_tensor("ln1_b", [DIM], f32, kind="ExternalInput")
    ln2_g = nc.dram_tensor("ln2_g", [DIM], f32, kind="ExternalInput")
    ln2_b = nc.dram_tensor("ln2_b", [DIM], f32, kind="ExternalInput")
    w1q = nc.dram_tensor("w1q", [LEVELS, DIM, HID], f8, kind="ExternalInput")
    cms_b1 = nc.dram_tensor("cms_b1", [LEVELS, HID], f32, kind="ExternalInput")
    w2q = nc.dram_tensor("w2q", [LEVELS, HID, DIM], f8, kind="ExternalInput")
    cms_b2 = nc.dram_tensor("cms_b2", [LEVELS, DIM], f32, kind="ExternalInput")
    maskT = nc.dram_tensor("maskT", [CHUNK, CHUNK], f32, kind="ExternalInput")
    out = nc.dram_tensor("out", [t_own, DIM], f32, kind="ExternalOutput")

    n_own_t = t_own // P
    n_blk = t_own // BLK
    n_pre_blk = t_pre // BLK
    ntt = BLK // P

    with tile.TileContext(nc) as tc, ExitStack() as top:
        dram = top.enter_context(tc.tile_pool(name="dram", bufs=1, space="DRAM"))
        x2_d = dram.tile([n_own_t, P, DIM], f32)

        consts = top.enter_context(tc.tile_pool(name="consts", bufs=1))
        ident_f = consts.tile([P, P], f32)
        make_identity(nc, ident_f)
        ident16 = consts.tile([P, P], f16)
        nc.vector.tensor_copy(out=ident16, in_=ident_f)
        eps_t = consts.tile([P, 1], f32)
        nc.vector.memset(eps_t, EPS)
        mask_t = consts.tile([CHUNK, CHUNK], f32)
        nc.sync.dma_start(out=mask_t, in_=maskT.ap())
        if fold_ln:
            g1 = b1 = g2 = b2 = None
        else:
            g1 = consts.tile([P, DIM], f32)
            b1 = consts.tile([P, DIM], f32)
            g2 = consts.tile([P, DIM], f32)
            b2 = consts.tile([P, DIM], f32)
            nc.sync.dma_start(out=g1, in_=ln1_g.ap()[None, :].partition_broadcast(P).opt())
            nc.sync.dma_start(out=b1, in_=ln1_b.ap()[None, :].partition_broadcast(P).opt())
            nc.sync.dma_start(out=g2, in_=ln2_g.ap()[None, :].partition_broadcast(P).opt())
            nc.sync.dma_start(out=b2, in_=ln2_b.ap()[None, :].partition_broadcast(P).opt())

        ln_w = top.enter_context(tc.tile_pool(name="ln_w", bufs=1))

        def layernorm(x_t, out_r, gain, bias, uid):
            """out_r = LN(x_t); gain/bias omitted when folded into weights."""
            BNF = nc.vector.BN_STATS_FMAX
            nsub = DIM // BNF
            stats = ln_w.tile([P, nsub, nc.vector.BN_STATS_DIM], f32, tag="ln_stats",
                              bufs=3, name=f"ln_st_{uid}")
            xg = x_t[:].rearrange("p (s f) -> p s f", f=BNF)
            for s_ in range(nsub):
                nc.vector.bn_stats(out=stats[:, s_, :], in_=xg[:, s_, :])
            mv = ln_w.tile([P, nc.vector.BN_AGGR_DIM], f32, tag="ln_mv", bufs=3,
                           name=f"ln_mv_{uid}")
            nc.vector.bn_aggr(out=mv, in_=stats)
            rstd = ln_w.tile([P, 1], f32, tag="ln_rstd", bufs=3, name=f"ln_rs_{uid}")
            nc.scalar.activation(out=rstd, in_=mv[:, 1:2], func=AF.Sqrt, bias=eps_t, scale=1.0)
            nc.vector.reciprocal(out=rstd, in_=rstd)
            if gain is None:
                nc.vector.tensor_scalar(out=out_r, in0=x_t, scalar1=mv[:, 0:1],
                                        scalar2=rstd, op0=ALU.subtract, op1=ALU.mult)
            else:
                tmp = ln_w.tile([P, DIM], f32, tag="ln_tmp", bufs=3, name=f"ln_tp_{uid}")
                nc.vector.tensor_scalar(out=tmp, in0=x_t, scalar1=mv[:, 0:1],
                                        scalar2=rstd, op0=ALU.subtract, op1=ALU.mult)
                nc.vector.tensor_mul(out=tmp, in0=tmp, in1=gain)
                nc.vector.tensor_add(out=out_r, in0=tmp, in1=bias)

        # hT_a: LN2(x2) transposed, fp8; written in phase B, FFN levels 0/2 input
        hTa_pool = top.enter_context(tc.tile_pool(name="hTa", bufs=1))
        hT_a = hTa_pool.tile([P, D_T, t_own], f8)

        # ---------------- attention (phases A+B) ----------------
        ab = ExitStack()
        mt_pool = ab.enter_context(tc.tile_pool(name="mt", bufs=1))
        # head h lives at [pb:pb+64, fi*64:(fi+1)*64], pb=(h%2)*64, fi=h//2
        Mt_f = mt_pool.tile([P, 512], f32)
        Mt_s = mt_pool.tile([P, 512], f16)
        nc.vector.memset(Mt_f, 0.0)
        wo_pool = ab.enter_context(tc.tile_pool(name="wo_pool", bufs=1))
        wo_all = wo.ap().rearrange("(kt p) d -> p kt d", p=P)
        wo_ts = []
        for nh in range(2):
            w_t = wo_pool.tile([P, D_T, 512], f16, tag=f"wo{nh}", name=f"wo_full_{nh}")
            nc.scalar.dma_start(out=w_t, in_=wo_all[:, :, nh * 512:(nh + 1) * 512])
            wo_ts.append(w_t)
        # own-token hT for all 4 blocks (computed in phase A)
        hTown_pool = ab.enter_context(tc.tile_pool(name="hTown", bufs=1))
        hTb_all = [hTown_pool.tile([P, D_T, BLK], f16, tag=f"hTb{b_}", name=f"hTb_{b_}")
                   for b_ in range(n_blk)]

        # ---------------- Phase A: prefix -> M, own-token LN1+transpose ----------
        with ExitStack() as pa:
            a_sb = pa.enter_context(tc.tile_pool(name="A_sb", bufs=1))
            a_ps = pa.enter_context(tc.tile_pool(name="A_ps", bufs=2, space="PSUM"))
            a_w = pa.enter_context(tc.tile_pool(name="A_w", bufs=2))

            def ln_transpose(src_dram, t_global, dst_slice_fn, tagp, uid):
                """DMA one 128-token tile, LN, PE-transpose into dst."""
                x_t = a_sb.tile([P, DIM], f32, tag=f"{tagp}_x", bufs=3, name=f"{tagp}_x_{uid}")
                nc.sync.dma_start(out=x_t, in_=src_dram.ap()[t_global * P:(t_global + 1) * P, :])
                h_r = a_sb.tile([P, DIM], f16, tag=f"{tagp}_h", bufs=2, name=f"{tagp}_h_{uid}")
                layernorm(x_t, h_r, g1, b1, f"{tagp}{uid}")
                for fidx in range(D_T):
                    tps = a_ps.tile([P, P], f16, tag="A_tp", name=f"{tagp}_tp_{uid}_{fidx}")
                    nc.tensor.transpose(tps, h_r[:, fidx * P:(fidx + 1) * P], ident16)
                    nc.scalar.copy(out=dst_slice_fn(fidx), in_=tps)

            for blk in range(n_pre_blk):
                hTp = a_sb.tile([P, D_T, BLK], f16, tag="A_hT", bufs=1, name=f"hTp_{blk}")
                for t in range(ntt):
                    ln_transpose(x_pre, blk * ntt + t,
                                 lambda fidx, t=t: hTp[:, fidx, t * P:(t + 1) * P],
                                 "A", f"{blk}_{t}")
                # own-token LN for block blk (hoisted from phase B)
                for t in range(ntt):
                    ln_transpose(x_own, blk * ntt + t,
                                 lambda fidx, t=t: hTb_all[blk][:, fidx, t * P:(t + 1) * P],
                                 "O", f"{blk}_{t}")
                kcp = a_sb.tile([P, ntt, DIM], f16, tag="A_kc", bufs=1, name=f"kcp_{blk}")
                vcp = a_sb.tile([P, ntt, DIM], f16, tag="A_vc", bufs=1, name=f"vcp_{blk}")
                for (w_in, dst) in ((wk, kcp), (wv, vcp)):
                    w_all = w_in.ap().rearrange("(kt p) d -> p kt d", p=P)
                    for nh in range(2):
                        w_t = a_w.tile([P, D_T, 512], f16, tag="A_wt", name=f"A_wt_{blk}_{dst.name}_{nh}")
                        nc.sync.dma_start(out=w_t, in_=w_all[:, :, nh * 512:(nh + 1) * 512])
                        for m in range(ntt):
                            pst = a_ps.tile([P, 512], f32, tag="A_pst", name=f"A_pst_{blk}_{dst.name}_{nh}_{m}")
                            for k in range(D_T):
                                nc.tensor.matmul(pst, hTp[:, k, m * P:(m + 1) * P], w_t[:, k, :],
                                                 start=(k == 0), stop=(k == D_T - 1))
                            nc.scalar.copy(out=dst[:, m, nh * 512:(nh + 1) * 512], in_=pst)
                dbank = a_ps.tile([P, 512], f32, tag="A_dm", name=f"A_dm_{blk}")
                for h in range(HEADS):
                    pb, fi = (h % 2) * HD, h // 2
                    reg = dbank[pb:pb + HD, fi * HD:(fi + 1) * HD]
                    for ch in range(ntt):
                        nc.tensor.matmul(reg, kcp[:, ch, h * HD:(h + 1) * HD],
                                         vcp[:, ch, h * HD:(h + 1) * HD],
                                         start=(ch == 0), stop=(ch == ntt - 1))
                nc.vector.tensor_add(out=Mt_f, in0=Mt_f, in1=dbank)
        nc.scalar.copy(out=Mt_s, in_=Mt_f)

        # ---------------- Phase B: projections, scan, attn out ----------------
        with ExitStack() as pbk:
            b_sb = pbk.enter_context(tc.tile_pool(name="B_sb", bufs=1))
            b_ps = pbk.enter_context(tc.tile_pool(name="B_ps", bufs=2, space="PSUM"))
            b_w = pbk.enter_context(tc.tile_pool(name="B_w", bufs=2))
            h2_pend = []   # (h2_r tile, ti) pending transposes, pipelined one block

            def flush_h2():
                for h2_r, ti in h2_pend:
                    for fidx in range(D_T):
                        tps = b_ps.tile([P, P], f16, tag="B_tp", name=f"B_h2tp_{ti}_{fidx}")
                        nc.tensor.transpose(tps, h2_r[:, fidx * P:(fidx + 1) * P], ident16)
                        nc.scalar.copy(out=hT_a[:, fidx, ti * P:(ti + 1) * P], in_=tps)
                h2_pend.clear()

            for blk in range(n_blk):
                tok0 = blk * BLK
                hTb = hTb_all[blk]
                flush_h2()
                qT = b_sb.tile([P, D_T, BLK], f16, tag="B_qT", bufs=1, name=f"qT_{blk}")
                kT = b_sb.tile([P, D_T, BLK], f16, tag="B_kT", bufs=1, name=f"kT_{blk}")
                for (w_in, dst) in ((wq, qT), (wk, kT)):
                    w_all = w_in.ap().rearrange("(kt p) d -> p kt d", p=P)
                    for m in range(D_T):
                        w_t = b_w.tile([P, D_T, P], f16, tag="B_wt", name=f"B_wt_{blk}_{dst.name}_{m}")
                        nc.sync.dma_start(out=w_t, in_=w_all[:, :, m * P:(m + 1) * P])
                        pst = b_ps.tile([P, BLK], f32, tag="B_pst", name=f"B_pst_{blk}_{dst.name}_{m}")
                        for k in range(D_T):
                            nc.tensor.matmul(pst, w_t[:, k, :], hTb[:, k, :],
                                             start=(k == 0), stop=(k == D_T - 1))
                        nc.scalar.copy(out=dst[:, m, :], in_=pst)
                kc = b_sb.tile([P, ntt, DIM], f16, tag="B_kc", bufs=1, name=f"kc_{blk}")
                v = b_sb.tile([P, ntt, DIM], f16, tag="B_v", bufs=1, name=f"v_{blk}")
                for (w_in, dst) in ((wk, kc), (wv, v)):
                    w_all = w_in.ap().rearrange("(kt p) d -> p kt d", p=P)
                    for nh in range(2):
                        w_t = b_w.tile([P, D_T, 512], f16, tag="B_wtv", name=f"B_wtv_{blk}_{dst.name}_{nh}")
                        nc.sync.dma_start(out=w_t, in_=w_all[:, :, nh * 512:(nh + 1) * 512])
                        for m in range(ntt):
                            pst = b_ps.tile([P, 512], f32, tag="B_pst", name=f"B_pstv_{blk}_{dst.name}_{nh}_{m}")
                            for k in range(D_T):
                                nc.tensor.matmul(pst, hTb[:, k, m * P:(m + 1) * P], w_t[:, k, :],
                                                 start=(k == 0), stop=(k == D_T - 1))
                            nc.scalar.copy(out=dst[:, m, nh * 512:(nh + 1) * 512], in_=pst)
                # scan: chunk-outer, head-inner; y pre-transposed; batched M commit
                yTb = b_sb.tile([P, D_T, BLK], f16, tag="B_yT", bufs=1, name=f"yTb_{blk}")
                for ch in range(ntt):
                    dbank = b_ps.tile([P, 512], f32, tag="B_dm", name=f"dm_{blk}_{ch}")
                    for h in range(HEADS):
                        pb, fi = (h % 2) * HD, h // 2
                        qcT = qT[pb:pb + HD, fi, ch * P:(ch + 1) * P]
                        kcT = kT[pb:pb + HD, fi, ch * P:(ch + 1) * P]
                        vc = v[:, ch, h * HD:(h + 1) * HD]
                        kc_s = kc[:, ch, h * HD:(h + 1) * HD]
                        scan_ps = b_ps.tile([P, 256], f32, tag="B_scan", bufs=2,
                                            name=f"scan_{blk}_{ch}_{h}")
                        sc_ps = scan_ps[:, 0:P]
                        yT_ps = scan_ps[0:HD, P:2 * P]
                        nc.tensor.matmul(sc_ps, kcT, qcT, start=True, stop=True)
                        sc_r = b_sb.tile([P, P], f16, tag="B_scr", bufs=3, name=f"scr_{blk}_{ch}_{h}")
                        nc.vector.tensor_mul(out=sc_r, in0=sc_ps, in1=mask_t)
                        nc.tensor.matmul(yT_ps, vc, sc_r, start=True, stop=False)
                        nc.tensor.matmul(yT_ps, Mt_s[pb:pb + HD, fi * HD:(fi + 1) * HD],
                                         qcT, start=False, stop=True)
                        nc.scalar.copy(out=yTb[pb:pb + HD, fi, ch * P:(ch + 1) * P], in_=yT_ps)
                        nc.tensor.matmul(dbank[pb:pb + HD, fi * HD:(fi + 1) * HD],
                                         kc_s, vc, start=True, stop=True)
                        if h == HEADS // 2 - 1:   # heads 0..7 done -> commit cols 0:256
                            nc.vector.tensor_add(out=Mt_f[:, 0:256], in0=Mt_f[:, 0:256],
                                                 in1=dbank[:, 0:256])
                            nc.scalar.copy(out=Mt_s[:, 0:256], in_=Mt_f[:, 0:256])
                    nc.vector.tensor_add(out=Mt_f[:, 256:512], in0=Mt_f[:, 256:512],
                                         in1=dbank[:, 256:512])
                    nc.scalar.copy(out=Mt_s[:, 256:512], in_=Mt_f[:, 256:512])
                # attn out + residual + LN2 (h2 transposes deferred one block)
                for m in range(ntt):
                    ti = (tok0 // P) + m
                    x_t = b_sb.tile([P, DIM], f32, tag="B_x2t", bufs=2, name=f"B_x2t_{blk}_{m}")
                    nc.sync.dma_start(out=x_t, in_=x_own.ap()[tok0 + m * P:tok0 + (m + 1) * P, :])
                    x2_t = b_sb.tile([P, DIM], f32, tag="B_x2", bufs=2, name=f"B_x2_{blk}_{m}")
                    for nh in range(2):
                        pst = b_ps.tile([P, 512], f32, tag="B_pst", name=f"B_at_{blk}_{m}_{nh}")
                        for k in range(D_T):
                            nc.tensor.matmul(pst, yTb[:, k, m * P:(m + 1) * P], wo_ts[nh][:, k, :],
                                             start=(k == 0), stop=(k == D_T - 1))
                        nc.vector.tensor_add(out=x2_t[:, nh * 512:(nh + 1) * 512],
                                             in0=x_t[:, nh * 512:(nh + 1) * 512], in1=pst)
                    nc.scalar.dma_start(out=x2_d[ti], in_=x2_t)
                    h2_r = b_sb.tile([P, DIM], f16, tag="B_h2", bufs=8, name=f"B_h2_{blk}_{m}")
                    layernorm(x2_t, h2_r, g2, b2, f"h2_{blk}_{m}")
                    h2_pend.append((h2_r, ti))
            flush_h2()
        ab.close()

        # ---------------- Phase C: CMS FFN, fp8 DoubleRow ----------------
        with ExitStack() as pc:
            ffn = pc.enter_context(tc.tile_pool(name="ffn", bufs=1))
            hT_b = ffn.tile([P, D_T, t_own], f8)
            ug = ffn.tile([P, H_T, t_own], f8)
            w2r = ffn.tile([P, H_T, DIM], f8)   # level-2 w2, resident
            w2r_all = w2q.ap()[LEVELS - 1].rearrange("(kt p) d -> p kt d", p=P)
            for kh in range(4):
                HK = H_T // 4
                nc.scalar.dma_start(out=w2r[:, kh * HK:(kh + 1) * HK, :],
                                    in_=w2r_all[:, kh * HK:(kh + 1) * HK, :])
            bias_p = pc.enter_context(tc.tile_pool(name="bias", bufs=1))
            b2_bc = bias_p.tile([P, DIM], f32)
            nc.sync.dma_start(out=b2_bc, in_=cms_b2.ap()[LEVELS - 1][None, :].partition_broadcast(P).opt())
            c_w = pc.enter_context(tc.tile_pool(name="C_w", bufs=3))

            hT_io = [(hT_a, hT_b), (hT_b, hT_a), (hT_a, None)]
            n_tt = t_own // 512
            for lvl in range(LEVELS):
                hT_in, hT_out = hT_io[lvl]
                b1_t = bias_p.tile([P, H_T], f32, tag="b1t", bufs=2, name=f"b1t_{lvl}")
                nc.sync.dma_start(out=b1_t, in_=cms_b1.ap()[lvl].rearrange("(m p) -> p m", p=P))
                if lvl < LEVELS - 1:
                    b2_t = bias_p.tile([P, D_T], f32, tag="b2t", bufs=2, name=f"b2t_{lvl}")
                    nc.sync.dma_start(out=b2_t, in_=cms_b2.ap()[lvl].rearrange("(m p) -> p m", p=P))
                w1_all = w1q.ap()[lvl].rearrange("(kt p) d -> p kt d", p=P)
                with ExitStack() as sub:
                    ps = sub.enter_context(tc.tile_pool(name=f"C{lvl}_up_ps", bufs=2, space="PSUM"))
                    for m in range(H_T):
                        w_t = c_w.tile([P, D_T, P], f8, tag="up_w", name=f"up_w_{lvl}_{m}")
                        nc.sync.dma_start(out=w_t, in_=w1_all[:, :, m * P:(m + 1) * P])
                        psl = [ps.tile([P, 512], f32, tag=f"up_ps{t}", name=f"up_ps_{lvl}_{m}_{t}")
                               for t in range(n_tt)]
                        for kp in range(D_T // 2):
                            for tt in range(n_tt):
                                nc.tensor.matmul(psl[tt], w_t[:, 2 * kp:2 * kp + 2, :],
                                                 hT_in[:, 2 * kp:2 * kp + 2, tt * 512:(tt + 1) * 512],
                                                 start=(kp == 0), stop=(kp == D_T // 2 - 1),
                                                 perf_mode=DR)
                        for tt in range(n_tt):
                            nc.scalar.activation(out=ug[:, m, tt * 512:(tt + 1) * 512], in_=psl[tt],
                                                 func=AF.Gelu_apprx_tanh,
                                                 bias=b1_t[:, m:m + 1], scale=1.0 / W_SCALE)
                if lvl < LEVELS - 1:
                    w2_all = w2q.ap()[lvl].rearrange("(kt p) d -> p kt d", p=P)
                    with ExitStack() as sub:
                        ps = sub.enter_context(tc.tile_pool(name=f"C{lvl}_dn_ps", bufs=2, space="PSUM"))
                        for m in range(D_T):
                            w_t = c_w.tile([P, H_T, P], f8, tag="dn_w", name=f"dn_w_{lvl}_{m}")
                            nc.sync.dma_start(out=w_t, in_=w2_all[:, :, m * P:(m + 1) * P])
                            psl = [ps.tile([P, 512], f32, tag=f"dn_ps{t}", name=f"dn_ps_{lvl}_{m}_{t}")
                                   for t in range(n_tt)]
                            for kp in range(H_T // 2):
                                for tt in range(n_tt):
                                    nc.tensor.matmul(psl[tt], w_t[:, 2 * kp:2 * kp + 2, :],
                                                     ug[:, 2 * kp:2 * kp + 2, tt * 512:(tt + 1) * 512],
                                                     start=(kp == 0), stop=(kp == H_T // 2 - 1),
                                                     perf_mode=DR)
                            for tt in range(n_tt):
                                nc.vector.tensor_scalar(out=hT_out[:, m, tt * 512:(tt + 1) * 512],
                                                        in0=psl[tt], scalar1=1.0 / W_SCALE,
                                                        scalar2=b2_t[:, m:m + 1],
                                                        op0=ALU.mult, op1=ALU.add)
                else:
                    with ExitStack() as sub:
                        ps = sub.enter_context(tc.tile_pool(name=f"C{lvl}_dn_ps", bufs=2, space="PSUM"))
                        o_sb = sub.enter_context(tc.tile_pool(name=f"C{lvl}_o", bufs=1))
                        for ti in range(n_own_t):
                            x2_t = o_sb.tile([P, DIM], f32, tag="D_x2", bufs=3, name=f"D_x2_{ti}")
                            nc.sync.dma_start(out=x2_t, in_=x2_d[ti])
                            x2b_t = o_sb.tile([P, DIM], f32, tag="D_x2b", bufs=3, name=f"D_x2b_{ti}")
                            nc.vector.tensor_add(out=x2b_t, in0=x2_t, in1=b2_bc)
                            psl = [ps.tile([P, 512], f32, tag=f"o_ps{nh}", name=f"o_ps_{ti}_{nh}")
                                   for nh in range(2)]
                            for kp in range(H_T // 2):
                                for nh in range(2):
                                    nc.tensor.matmul(psl[nh], ug[:, 2 * kp:2 * kp + 2, ti * P:(ti + 1) * P],
                                                     w2r[:, 2 * kp:2 * kp + 2, nh * 512:(nh + 1) * 512],
                                                     start=(kp == 0), stop=(kp == H_T // 2 - 1),
                                                     perf_mode=DR)
                            o_t = o_sb.tile([P, DIM], f32, tag="D_o", bufs=3, name=f"D_o_{ti}")
                            for nh in range(2):
                                sl = slice(nh * 512, (nh + 1) * 512)
                                nc.scalar.activation(out=o_t[:, sl], in_=psl[nh], func=AF.Copy,
                                                     bias=0.0, scale=1.0 / W_SCALE)
                                nc.vector.tensor_add(out=o_t[:, sl], in0=o_t[:, sl], in1=x2b_t[:, sl])
                            nc.scalar.dma_start(out=out.ap()[ti * P:(ti + 1) * P, :], in_=o_t)

    _split_multi_waits(nc)
    return nc


_NC_CACHE = {}
LAST_RESULT = None


def _get_nc(key, **kw):
    if key not in _NC_CACHE:
        _NC_CACHE[key] = build_kernel(**kw)
    return _NC_CACHE[key]


def _q8(w, scale=W_SCALE):
    return np.clip(np.asarray(w, np.float32) * scale, -240.0, 240.0).astype(ml_dtypes.float8_e4m3)


def kernel(x, ln1_g, ln1_b, wq, wk, wv, wo, ln2_g, ln2_b,
           cms_w1, cms_b1, cms_w2, cms_b2, **extra):
    x = np.asarray(x, np.float32)
    ln1_g, ln1_b = np.asarray(ln1_g, np.float32), np.asarray(ln1_b, np.float32)
    ln2_g, ln2_b = np.asarray(ln2_g, np.float32), np.asarray(ln2_b, np.float32)
    wq, wk, wv, wo = (np.asarray(w, np.float32) for w in (wq, wk, wv, wo))
    cms_w1 = np.asarray(cms_w1, np.float32)
    cms_b1 = np.asarray(cms_b1, np.float32)
    fold = float(np.abs(ln1_b).max()) == 0.0 and float(np.abs(ln2_b).max()) == 0.0
    if fold:
        # exact: fold LN gains into the consuming weight rows
        wq = ln1_g[:, None] * wq
        wk = ln1_g[:, None] * wk
        wv = ln1_g[:, None] * wv
        cms_w1 = cms_w1.copy()
        cms_w1[0] = ln2_g[:, None] * cms_w1[0]
    maskT = np.triu(np.ones((CHUNK, CHUNK), np.float32))  # maskT[e,c] = e<=c
    common = {
        "wq": wq.astype(np.float16), "wk": wk.astype(np.float16),
        "wv": wv.astype(np.float16), "wo": wo.astype(np.float16),
        "ln1_g": ln1_g, "ln1_b": ln1_b, "ln2_g": ln2_g, "ln2_b": ln2_b,
        "w1q": _q8(cms_w1), "cms_b1": cms_b1,
        "w2q": _q8(cms_w2), "cms_b2": np.asarray(cms_b2, np.float32),
        "maskT": maskT,
    }
    zeros_pre = np.zeros((T_PRE, DIM), np.float32)
    in_maps = []
    for c in range(N_CORES):
        b, half = c // 2, c % 2
        own = x[b, half * T_OWN:(half + 1) * T_OWN]
        pre = x[b, 0:T_PRE] if half else zeros_pre
        in_maps.append({**common, "x_own": np.ascontiguousarray(own),
                        "x_pre": np.ascontiguousarray(pre)})
    nc = _get_nc(("fold", fold), fold_ln=fold)
    res = run_bass_kernel_spmd(nc, in_maps, core_ids=list(range(N_CORES)))
    global LAST_RESULT
    LAST_RESULT = res
    out = np.empty((B, S, DIM), np.float32)
    for c in range(N_CORES):
        b, half = c // 2, c % 2
        out[b, half * T_OWN:(half + 1) * T_OWN] = res.results[c]["out"]
    return out
